# revision 6
# baseline (speedup 1.0000x reference)
"""LGCN encoder (3-layer LightGCN propagation) on 8 Trainium2 NeuronCores.

Strategy (1D row partition, SPMD single program):
  - Nodes padded to 150528 = 8 shards x 18816 rows (147 blocks x 128).
  - Per core: its shard's edges, sorted by (col-page, 128-row group, 32-row
    window). Gathers h[col] from a replicated HBM table via the custom
    SWDGE dma_gather (256B/row descriptors, 4 queues round-robin).
  - Scale-by-val + segment-sum fused into TensorE matmuls:
      psum[32-row window, 64] += S[128 edges, 32].T @ V[128 edges, 64]
    with S = one-hot(row-in-window) * val, host-prebuilt, streamed.
  - Four windows share one [128,64] psum tile via 32-aligned partition
    slices; DVE flushes each (page x group) into the SBUF h accumulator.
  - Per layer: AllGather of the 18816-row shard rebuilds the full table.
  - Output = (ego + h1 + h2 + h3)/4, reassembled host-side.

The chunk grid (chunks per (page, group, window) cell) is the max over the
8 cores, so one program serves all cores; padding slots gather node 0 with
S=0.
"""
import numpy as np

USER_NUM = 100000
ITEM_NUM = 50000
N_NODES = USER_NUM + ITEM_NUM
EMB = 64
N_LAYERS = 3
NCORES = 8

SHARD = 18816            # rows per core = 147 * 128
PAD_N = SHARD * NCORES   # 150528
GROUPS = SHARD // 128    # 147
W = 32                   # psum window rows
WINS = 4                 # windows per group
PAGE_W = 32768           # gather page width (int16 index reach)
NPAGES = (PAD_N + PAGE_W - 1) // PAGE_W  # 5
CHUNK = 128              # edges per matmul
CALL_CHUNKS = 32         # chunks per dma_gather call (4096 idxs)
CALL_IDX = CALL_CHUNKS * CHUNK

_cache = {}


def _host_prep(adj_row, adj_col, adj_vals):
    """Build per-core idx/S streams + the shared chunk grid."""
    row = adj_row.astype(np.int64)
    col = adj_col.astype(np.int64)
    vals = adj_vals.astype(np.float32)
    core = row // SHARD
    page = np.minimum(col // PAGE_W, NPAGES - 1)
    rl = row - core * SHARD                      # row local to shard
    group = rl >> 7
    win = (rl >> 5) & 3
    rowl = rl & 31
    lcol = col - page * PAGE_W                   # int16-safe local col
    # cell id in stream order: (page, group, win)
    cell = (page * GROUPS + group) * WINS + win
    NCELLS = NPAGES * GROUPS * WINS

    # per-core per-cell counts -> shared chunk grid
    counts = np.zeros((NCORES, NCELLS), dtype=np.int64)
    for c in range(NCORES):
        m = core == c
        counts[c] = np.bincount(cell[m], minlength=NCELLS)
    n_chunks = np.maximum(1, -(-counts.max(axis=0) // CHUNK))  # >=1 per cell

    # page-major chunk stream with per-page padding to CALL_CHUNKS multiples
    cells_per_page = GROUPS * WINS
    chunk_base = np.zeros(NCELLS, dtype=np.int64)
    chunk_meta = []   # (win, is_start, is_stop, flush_group_or_-1, page)
    call_meta = []    # (page, first_chunk_global)
    tot_chunks = 0
    for p in range(NPAGES):
        page_first_chunk = tot_chunks
        for g in range(GROUPS):
            for w in range(WINS):
                cid = (p * GROUPS + g) * WINS + w
                chunk_base[cid] = tot_chunks
                nc_ = int(n_chunks[cid])
                for k in range(nc_):
                    flush = g if (w == WINS - 1 and k == nc_ - 1) else -1
                    chunk_meta.append((w, k == 0, k == nc_ - 1, flush, p))
                tot_chunks += nc_
        # pad page to call multiple with dummy chunks (no matmul)
        while (tot_chunks - page_first_chunk) % CALL_CHUNKS:
            chunk_meta.append((-1, False, False, -1, p))
            tot_chunks += 1
        for j in range((tot_chunks - page_first_chunk) // CALL_CHUNKS):
            call_meta.append((p, page_first_chunk + j * CALL_CHUNKS))

    tot_slots = tot_chunks * CHUNK

    # per-core slot assignment
    idx_streams, s_streams = [], []
    for c in range(NCORES):
        m = core == c
        cc, ll, rr, vv = cell[m], lcol[m], rowl[m], vals[m]
        order = np.argsort(cc, kind="stable")
        cc, ll, rr, vv = cc[order], ll[order], rr[order], vv[order]
        # rank within cell
        uniq, first_pos, cnts = np.unique(cc, return_index=True, return_counts=True)
        rank = np.arange(cc.size) - np.repeat(first_pos, cnts)
        slot = chunk_base[cc] * CHUNK + rank
        idx_flat = np.zeros(tot_slots, dtype=np.int16)
        idx_flat[slot] = ll.astype(np.int16)
        sflat = np.zeros((tot_slots, W), dtype=np.float32)
        sflat[slot, rr] = vv
        # wrap idxs [128, tot_slots/16] (16-partition pattern replicated x8)
        base16 = idx_flat.reshape(-1, 16).T          # [16, S]
        idx_w = np.tile(base16, (8, 1))              # [128, S]
        # S partition-major [128, tot_chunks, W]
        s_pm = np.ascontiguousarray(
            sflat.reshape(tot_chunks, CHUNK, W).transpose(1, 0, 2))
        idx_streams.append(np.ascontiguousarray(idx_w))
        s_streams.append(s_pm)

    return idx_streams, s_streams, chunk_meta, call_meta, tot_chunks


def _build_program(chunk_meta, call_meta, tot_chunks):
    import concourse.bacc as bacc
    import concourse.tile as tile
    import concourse.mybir as mybir
    from concourse.library_config import mlp

    f32 = mybir.dt.float32
    nc = bacc.Bacc("TRN2", target_bir_lowering=False, debug=False,
                   num_devices=NCORES, num_swdge_queues=4)

    table0 = nc.dram_tensor("table0", [PAD_N, EMB], f32, kind="ExternalInput")
    ego_d = nc.dram_tensor("ego", [128, GROUPS, EMB], f32, kind="ExternalInput")
    idx_d = nc.dram_tensor("idxs", [128, tot_chunks * CHUNK // 16],
                           mybir.dt.int16, kind="ExternalInput")
    s_d = nc.dram_tensor("smat", [128, tot_chunks, W], f32, kind="ExternalInput")
    out_d = nc.dram_tensor("out", [128, GROUPS, EMB], f32, kind="ExternalOutput")

    ag_in = [nc.dram_tensor(f"ag_in{l}", [SHARD, EMB], f32) for l in range(N_LAYERS - 1)]
    ag_out = [nc.dram_tensor(f"ag_out{l}", [PAD_N, EMB], f32) for l in range(N_LAYERS - 1)]

    page_bounds = [(p * PAGE_W, min((p + 1) * PAGE_W, PAD_N)) for p in range(NPAGES)]

    with tile.TileContext(nc) as tc:
        nc.gpsimd.load_library(mlp)
        with (
            tc.tile_pool(name="persist", bufs=1) as persist,
            tc.tile_pool(name="vpool", bufs=6) as vpool,
            tc.tile_pool(name="spool", bufs=6) as spool,
            tc.tile_pool(name="ipool", bufs=6) as ipool,
            tc.tile_pool(name="psum", bufs=4, space="PSUM") as psum_pool,
        ):
            acc = persist.tile([128, GROUPS, EMB], f32)
            nc.sync.dma_start(acc[:], ego_d[:])
            h_t = persist.tile([128, GROUPS, EMB], f32)

            qctr = 0
            for l in range(N_LAYERS):
                src = table0 if l == 0 else ag_out[l - 1]
                nc.vector.memset(h_t[:], 0.0)

                ps = None
                for ci, (p, c0) in enumerate(call_meta):
                    lo, hi = page_bounds[p]
                    v_t = vpool.tile([128, CALL_CHUNKS, EMB], f32)
                    i_t = ipool.tile([128, CALL_IDX // 16], mybir.dt.int16)
                    nc.sync.dma_start(
                        i_t[:], idx_d[:, c0 * CHUNK // 16:(c0 + CALL_CHUNKS) * CHUNK // 16])
                    s_t = spool.tile([128, CALL_CHUNKS, W], f32)
                    nc.sync.dma_start(s_t[:], s_d[:, c0:c0 + CALL_CHUNKS, :])
                    nc.gpsimd.dma_gather(
                        v_t[:], src[lo:hi, :], i_t[:], CALL_IDX, CALL_IDX, EMB,
                        single_packet=False, queue_num=qctr % 4)
                    qctr += 1

                    for k in range(CALL_CHUNKS):
                        ch = c0 + k
                        w, is_start, is_stop, flush_g, pg = chunk_meta[ch]
                        if w < 0:
                            continue  # dummy pad chunk
                        if is_start and w == 0:
                            # new (page, group): fresh psum tile
                            ps = psum_pool.tile([128, EMB], f32)
                        nc.tensor.matmul(
                            ps[w * W:(w + 1) * W, :],
                            s_t[:, k, :],
                            v_t[:, k, :],
                            start=is_start, stop=is_stop,
                            tile_position=(0, w * W),
                        )
                        if flush_g >= 0:
                            nc.vector.tensor_add(
                                h_t[:, flush_g, :], h_t[:, flush_g, :], ps[:])

                nc.vector.tensor_add(acc[:], acc[:], h_t[:])
                if l < N_LAYERS - 1:
                    ag_view = ag_in[l].ap().rearrange("(b p) e -> p b e", p=128)
                    nc.sync.dma_start(ag_view, h_t[:])
                    nc.gpsimd.collective_compute(
                        "AllGather",
                        mybir.AluOpType.bypass,
                        replica_groups=[list(range(NCORES))],
                        ins=[ag_in[l].ap().opt()],
                        outs=[ag_out[l].ap().opt()],
                    )

            # h_t is free after the last layer; reuse it as output staging
            nc.scalar.mul(h_t[:], acc[:], 1.0 / (N_LAYERS + 1))
            nc.sync.dma_start(out_d[:], h_t[:])

    nc.compile()
    return nc


def _get_runner(chunk_meta, call_meta, tot_chunks):
    import time
    key = ("prog", tot_chunks, len(call_meta))
    if key in _cache:
        return _cache[key]
    t0 = time.time()
    nc = _build_program(chunk_meta, call_meta, tot_chunks)
    t1 = time.time()
    from axon_timing import build_runner
    run, _ = build_runner(nc, NCORES)
    _cache[key] = run
    print(f"[kernel] program build+compile: {t1-t0:.1f}s")
    return run


def kernel(user_emb, item_emb, adj_row, adj_col, adj_vals):
    ego = np.concatenate([np.asarray(user_emb), np.asarray(item_emb)], axis=0)
    ego_pad = np.zeros((PAD_N, EMB), dtype=np.float32)
    ego_pad[:N_NODES] = ego

    idx_streams, s_streams, chunk_meta, call_meta, tot_chunks = _host_prep(
        np.asarray(adj_row), np.asarray(adj_col), np.asarray(adj_vals))

    run = _get_runner(chunk_meta, call_meta, tot_chunks)

    in_maps = []
    for c in range(NCORES):
        shard = ego_pad[c * SHARD:(c + 1) * SHARD]
        ego_blk = np.ascontiguousarray(
            shard.reshape(GROUPS, 128, EMB).transpose(1, 0, 2))
        in_maps.append({
            "table0": ego_pad,
            "ego": ego_blk,
            "idxs": idx_streams[c],
            "smat": s_streams[c],
        })

    results, wall = run(in_maps, reps=1)
    kernel.last_wall_seconds = wall

    full = np.empty((PAD_N, EMB), dtype=np.float32)
    for c in range(NCORES):
        blk = results[c]["out"]  # [128, GROUPS, EMB]
        full[c * SHARD:(c + 1) * SHARD] = (
            blk.transpose(1, 0, 2).reshape(SHARD, EMB))
    return full[:USER_NUM], full[USER_NUM:N_NODES]
